# revision 1
# baseline (speedup 1.0000x reference)
"""Trainium2 Bass kernel: full (non-causal) softmax attention.

Input:  query/key/value [1, 4096, 16, 128] f32 (B, S, H, D).
Output: [1, 4096, 16, 128] f32 = softmax(Q K^T / sqrt(D)) V per head.

Sharding: 16 heads over 8 cores -> 2 heads per core, no collectives.
Host pre-transposes Q,K per head to [D, S]; the device returns the
UN-normalized attention output transposed [D, S] plus the softmax
denominator row [S]; the host does the final divide (cheap numpy).

Device pipeline, per head, per query-chunk QC (1024 queries):
  for kt in 32 key-chunks (128 keys each):
    ST[kt] = scores^T chunk: psum[128k, QCq]  (two N=512 fp32r matmuls,
             stationary KT chunk; moving operand = QT columns)
    PT[kt] = exp(ST / sqrt(128))              (ACT, psum->sbuf, fp32r)
    OUT   += V_kt^T @ PT[kt]                  (accumulating fp32r matmuls)
    den   += colsum(PT[kt])   split across PE (ones-vector matmuls),
             DVE and GPSIMD (tensor adds) to balance engine load
ACT (exp) is the throughput floor (~280us); everything else is tuned
to stay below it.
"""

import os
import sys
from contextlib import ExitStack

import numpy as np

sys.path.insert(0, "/opt/trn_rl_repo")

import concourse.bacc as bacc
import concourse.bass as bass
import concourse.tile as tile
from concourse import mybir
from concourse.bass_utils import run_bass_kernel_spmd

N_CORES = 8
S = 4096
H = 16
D = 128
HEADS_PER_CORE = H // N_CORES  # 2
KT_CHUNK = 128                  # keys per score tile (psum partition dim)
QC = 1024                       # queries per super-chunk (ACT tile free dim)
NMM = 512                       # moving free dim per matmul (psum bank, fp32 max)
SCALE = float(D) ** -0.5

F32 = mybir.dt.float32
F32R = mybir.dt.float32r

# per-32-chunk denominator-reduction role assignment (tuned from trace):
# 'P' = PE ones-matmul, 'V' = DVE tensor add, 'G' = GPSIMD tensor add
N_PE_DEN = 5
N_DVE_DEN = 27


def _den_roles(n_kt):
    roles = []
    for kt in range(n_kt):
        r = kt % 32
        if r < N_PE_DEN:
            roles.append("P")
        elif r < N_PE_DEN + N_DVE_DEN:
            roles.append("V")
        else:
            roles.append("G")
    # interleave so same-engine work is spread across the loop
    order = sorted(range(n_kt), key=lambda i: (i * 13) % n_kt)
    out = [None] * n_kt
    for slot, role in zip(order, roles):
        out[slot] = role
    return out


def build_program(s=S, heads=HEADS_PER_CORE):
    nc = bacc.Bacc("TRN2", target_bir_lowering=False, debug=False,
                   num_devices=N_CORES)

    n_kt = s // KT_CHUNK
    n_qc = s // QC
    roles = _den_roles(n_kt)

    qt_d = nc.dram_tensor("qt", [heads, D, s], F32, kind="ExternalInput")
    kt_d = nc.dram_tensor("kt", [heads, D, s], F32, kind="ExternalInput")
    v_d = nc.dram_tensor("v", [heads, s, D], F32, kind="ExternalInput")
    out_d = nc.dram_tensor("out", [heads, D, s], F32, kind="ExternalOutput")
    den_d = nc.dram_tensor("den", [heads, s], F32, kind="ExternalOutput")

    with tile.TileContext(nc) as tc, ExitStack() as ctx:
        consts = ctx.enter_context(tc.tile_pool(name="consts", bufs=1))
        qkv_pool = ctx.enter_context(tc.tile_pool(name="qkv", bufs=2))
        pt_pool = ctx.enter_context(tc.tile_pool(name="pt", bufs=10))
        acc_pool = ctx.enter_context(tc.tile_pool(name="acc", bufs=2))
        osb_pool = ctx.enter_context(tc.tile_pool(name="osb", bufs=3))
        densb_pool = ctx.enter_context(tc.tile_pool(name="densb", bufs=2))
        st_pool = ctx.enter_context(
            tc.tile_pool(name="st", bufs=2, space="PSUM"))
        outp_pool = ctx.enter_context(
            tc.tile_pool(name="outp", bufs=2, space="PSUM"))
        denp_pool = ctx.enter_context(
            tc.tile_pool(name="denp", bufs=1, space="PSUM"))

        ones_f = consts.tile([128, 1], F32, tag="ones_f")
        nc.vector.memset(ones_f[:], 1.0)
        ones_col = consts.tile([128, 1], F32R, tag="ones_col")
        nc.scalar.copy(ones_col[:], ones_f[:])

        # Per-head on-chip tensors (double-buffered across heads).
        # float32r tiles are bit-identical to f32; typing the producers
        # f32r keeps the BIR verifier happy for fp32r matmul consumers.
        def load_head(h):
            qt_sb = qkv_pool.tile([D, s], F32R, tag="qt")
            nc.sync.dma_start(out=qt_sb[:], in_=qt_d[h].bitcast(F32R))
            kt_sb = qkv_pool.tile([D, s], F32R, tag="kt")
            nc.sync.dma_start(out=kt_sb[:], in_=kt_d[h].bitcast(F32R))
            v_sb = qkv_pool.tile([128, n_kt, D], F32R, tag="v")
            nc.sync.dma_start(
                out=v_sb[:],
                in_=v_d[h].rearrange("(c p) d -> p c d", p=128).bitcast(F32R))
            return qt_sb, kt_sb, v_sb

        heads_sb = [load_head(0)]

        # Deferred epilogue work, interleaved into the next chunk's matmul
        # stream so the PE pipeline never waits on DVE.
        pending = []

        for h in range(heads):
            qt_sb, kt_sb, v_sb = heads_sb[h]
            if h + 1 < heads:
                heads_sb.append(load_head(h + 1))
            for qc in range(n_qc):
                q0 = qc * QC
                out_ps = [outp_pool.tile([D, NMM], F32, tag="outp",
                                         name=f"out_ps{j}")
                          for j in range(QC // NMM)]
                den_ps = denp_pool.tile([1, QC], F32, tag="denp")
                accs = {"V": [], "G": []}
                den_started = [False] * (QC // NMM)
                for kt in range(n_kt):
                    k0 = kt * KT_CHUNK
                    st = st_pool.tile([128, QC], F32, tag="st")
                    lhs_k = kt_sb[:, k0:k0 + KT_CHUNK]
                    for j in range(QC // NMM):
                        nc.tensor.matmul(
                            st[:, j * NMM:(j + 1) * NMM],
                            lhs_k,
                            qt_sb[:, q0 + j * NMM:q0 + (j + 1) * NMM],
                            start=True, stop=True)
                    pt = pt_pool.tile([128, QC], F32R, tag="pt")
                    nc.scalar.activation(
                        pt[:], st[:], mybir.ActivationFunctionType.Exp,
                        scale=SCALE)
                    lhs_v = v_sb[:, kt, :]
                    for j in range(QC // NMM):
                        nc.tensor.matmul(
                            out_ps[j][:],
                            lhs_v,
                            pt[:, j * NMM:(j + 1) * NMM],
                            start=(kt == 0), stop=(kt == n_kt - 1))
                    # softmax denominator partial reduction.
                    role = roles[kt]
                    if role == "P":
                        for j in range(QC // NMM):
                            nc.tensor.matmul(
                                den_ps[:, j * NMM:(j + 1) * NMM],
                                ones_col[:],
                                pt[:, j * NMM:(j + 1) * NMM],
                                start=(not den_started[j]), stop=False,
                                skip_group_check=True)
                            den_started[j] = True
                    else:
                        eng = nc.vector if role == "V" else nc.gpsimd
                        ptf = pt[:].bitcast(F32)
                        lst = accs[role]
                        if not lst:
                            a = acc_pool.tile([128, QC], F32, tag="acc" + role,
                                              name="acc" + role)
                            eng.tensor_copy(a[:], ptf)
                            lst.append(a)
                        else:
                            b = acc_pool.tile([128, QC], F32,
                                              tag="acc" + role + "b",
                                              name="acc" + role + "b")
                            eng.tensor_add(b[:], lst[-1][:], ptf)
                            lst.append(b)
                    if pending:
                        pending.pop(0)()

                def finish(out_ps=out_ps, den_ps=den_ps, accs=accs, h=h,
                           q0=q0, den_started=den_started):
                    folds = []
                    if accs["V"] and accs["G"]:
                        accm = acc_pool.tile([128, QC], F32, tag="accm")
                        nc.vector.tensor_add(accm[:], accs["V"][-1][:],
                                             accs["G"][-1][:])
                        folds.append(accm)
                    elif accs["V"] or accs["G"]:
                        folds.append((accs["V"] or accs["G"])[-1])
                    den_sb = densb_pool.tile([1, QC], F32, tag="den_sb")
                    out_sb = osb_pool.tile([D, QC], F32, tag="out_sb")

                    def s1():
                        # fold the DVE/GPSIMD accumulators into the psum
                        # denominator row (plain fp32 matmul: f32 producer).
                        started = list(den_started)
                        for fi, acc in enumerate(folds):
                            last = fi == len(folds) - 1
                            for j in range(QC // NMM):
                                nc.tensor.matmul(
                                    den_ps[:, j * NMM:(j + 1) * NMM],
                                    ones_f[:],
                                    acc[:, j * NMM:(j + 1) * NMM],
                                    start=(not started[j]), stop=last,
                                    skip_group_check=True)
                                started[j] = True

                    def s2():
                        nc.vector.tensor_copy(den_sb[:], den_ps[:])
                        nc.sync.dma_start(
                            out=den_d[h:h + 1, q0:q0 + QC], in_=den_sb[:])
                        for j in range(QC // NMM):
                            nc.vector.tensor_copy(
                                out_sb[:, j * NMM:(j + 1) * NMM],
                                out_ps[j][:])
                        nc.sync.dma_start(
                            out=out_d[h][:, q0:q0 + QC], in_=out_sb[:])

                    return [s1, s2]

                pending.extend(finish())
        while pending:
            pending.pop(0)()

    nc.compile()
    return nc


def _install_ntff_hook():
    """Provide antenv.axon_hooks (absent in this image) so that
    run_bass_kernel_spmd(trace=True) can capture NTFF profiles via the
    axon .so — mirrors trn_agent_boot.trn_boot._ntff_profile_via_ctypes."""
    try:
        from antenv.axon_hooks import get_axon_ntff_profile_hook  # noqa: F401
        return
    except ImportError:
        pass
    import contextlib
    import ctypes
    import types

    so_path = "/opt/axon/libaxon_pjrt.so"
    lib = ctypes.CDLL(so_path)
    if not hasattr(lib, "axon_start_nrt_profile"):
        return
    lib.axon_start_nrt_profile.argtypes = [
        ctypes.POINTER(ctypes.c_int64), ctypes.c_size_t]
    lib.axon_start_nrt_profile.restype = ctypes.c_int64
    lib.axon_stop_nrt_profile.argtypes = [ctypes.c_char_p]
    lib.axon_stop_nrt_profile.restype = ctypes.c_int64

    @contextlib.contextmanager
    def _hook(output_dir, device_ids):
        import jax
        jax.devices()
        if device_ids:
            ids = (ctypes.c_int64 * len(device_ids))(*device_ids)
            rc = lib.axon_start_nrt_profile(ids, len(device_ids))
        else:
            rc = lib.axon_start_nrt_profile(None, 0)
        if rc != 0:
            raise RuntimeError(f"axon_start_nrt_profile rc={rc}")
        try:
            yield
        finally:
            n = lib.axon_stop_nrt_profile(str(output_dir).encode())
            print(f"ntff profile: {n} file(s) written to {output_dir}")

    mod = types.ModuleType("antenv.axon_hooks")
    mod.get_axon_ntff_profile_hook = lambda: _hook
    mod.set_axon_ntff_profile_hook = lambda h: None
    import antenv
    sys.modules["antenv.axon_hooks"] = mod
    antenv.axon_hooks = mod


_CACHE = {}


def _get_program():
    key = "main"
    if key not in _CACHE:
        _CACHE[key] = build_program()
    return _CACHE[key]


def kernel(query, key, value, trace=False, **trace_kwargs):
    assert query.shape == (1, S, H, D)
    nc = _get_program()

    q = np.asarray(query, dtype=np.float32)[0]   # [S, H, D]
    k = np.asarray(key, dtype=np.float32)[0]
    v = np.asarray(value, dtype=np.float32)[0]

    in_maps = []
    for c in range(N_CORES):
        hs = slice(c * HEADS_PER_CORE, (c + 1) * HEADS_PER_CORE)
        # [S, h, D] -> [h, D, S]
        qt = np.ascontiguousarray(q[:, hs, :].transpose(1, 2, 0))
        kt = np.ascontiguousarray(k[:, hs, :].transpose(1, 2, 0))
        vv = np.ascontiguousarray(v[:, hs, :].transpose(1, 0, 2))
        in_maps.append({"qt": qt, "kt": kt, "v": vv})

    if trace:
        _install_ntff_hook()
    res = run_bass_kernel_spmd(nc, in_maps, core_ids=list(range(N_CORES)),
                               trace=trace, **trace_kwargs)

    out = np.empty((1, S, H, D), dtype=np.float32)
    for c in range(N_CORES):
        o = res.results[c]["out"]    # [h, D, S] unnormalized
        den = res.results[c]["den"]  # [h, S]
        for i in range(HEADS_PER_CORE):
            out[0, :, c * HEADS_PER_CORE + i, :] = (o[i] / den[i][None, :]).T
    if trace:
        kernel.last_results = res
    return out



# revision 2
# speedup vs baseline: 1.2747x; 1.2747x over previous
"""Trainium2 Bass kernel: full (non-causal) softmax attention.

Input:  query/key/value [1, 4096, 16, 128] f32 (B, S, H, D).
Output: [1, 4096, 16, 128] f32 = softmax(Q K^T / sqrt(D)) V per head.

Sharding: 16 heads over 8 cores -> 2 heads per core, no collectives.
Host pre-transposes Q,K per head to [D, S] in fp16; the device returns
the UN-normalized attention output [D, 512] per (head, q-chunk) plus
fp16 partial denominator accumulators [128, 4, 512]; the host reduces
the accumulators (sum over 128 k-lanes x 4 slots) and does the final
divide (cheap numpy).

Device pipeline (ACT-exp is the throughput floor, ~246us/core):
  - global stream of 512 score chunks (2 heads x 8 q-chunks x 32 kt),
    grouped into alternating 4-bank / 3-bank PSUM super-tiles
  - per group: PE writes scores (fp16 matmuls, N=512/bank), one big
    ACT exp (N=2048/1536, fp32 psum -> fp16 sbuf), then PE PV matmuls
    accumulate into a single out bank; DVE accumulates the softmax
    denominator with fp16 2x-mode adds into per-job accumulators.
  - software-pipelined by one group so PE never waits on ACT.
"""

import sys
from contextlib import ExitStack

import numpy as np

sys.path.insert(0, "/opt/trn_rl_repo")

import concourse.bacc as bacc
import concourse.bass as bass
import concourse.tile as tile
from concourse import mybir
from concourse.bass_utils import run_bass_kernel_spmd

N_CORES = 8
S = 4096
H = 16
D = 128
HPC = H // N_CORES   # heads per core = 2
QC = 512             # queries per job (one psum bank of fp32)
NQC = S // QC        # 8 q-chunks per head
KT = 128             # keys per score chunk (psum partition dim)
NKT = S // KT        # 32 key chunks
SCALE = float(D) ** -0.5
GA = 4               # chunks per group in buffer A (4 psum banks)
GB = 3               # chunks per group in buffer B (3 psum banks)

F32 = mybir.dt.float32
F16 = mybir.dt.float16


def _make_groups():
    """Global chunk stream split into alternating A/B groups."""
    chunks = [(h, qc, kt)
              for h in range(HPC) for qc in range(NQC) for kt in range(NKT)]
    groups = []
    i = 0
    use_a = True
    while i < len(chunks):
        n = min(GA if use_a else GB, len(chunks) - i)
        groups.append((use_a, chunks[i:i + n]))
        i += n
        use_a = not use_a
    return groups


def build_program():
    nc = bacc.Bacc("TRN2", target_bir_lowering=False, debug=False,
                   num_devices=N_CORES)

    qt_d = nc.dram_tensor("qt", [HPC, D, S], F16, kind="ExternalInput")
    kt_d = nc.dram_tensor("kt", [HPC, D, S], F16, kind="ExternalInput")
    v_d = nc.dram_tensor("v", [HPC, 128, NKT, D], F16, kind="ExternalInput")
    out_d = nc.dram_tensor("out", [HPC, NQC, D, QC], F32,
                           kind="ExternalOutput")
    acc_d = nc.dram_tensor("acc", [HPC, NQC, 128, GA, QC], F16,
                           kind="ExternalOutput")

    groups = _make_groups()

    with tile.TileContext(nc) as tc, ExitStack() as ctx:
        consts = ctx.enter_context(tc.tile_pool(name="consts", bufs=1))
        qkv_pool = ctx.enter_context(tc.tile_pool(name="qkv", bufs=2))
        pt_pool = ctx.enter_context(tc.tile_pool(name="pt", bufs=4))
        acc_pool = ctx.enter_context(tc.tile_pool(name="acc", bufs=2))
        osb_pool = ctx.enter_context(tc.tile_pool(name="osb", bufs=3))
        stA_pool = ctx.enter_context(
            tc.tile_pool(name="stA", bufs=1, space="PSUM"))
        stB_pool = ctx.enter_context(
            tc.tile_pool(name="stB", bufs=1, space="PSUM"))
        outp_pool = ctx.enter_context(
            tc.tile_pool(name="outp", bufs=1, space="PSUM"))

        # ---- warmup: ACT table load + PE HAM ramp while DMAs run ----
        wsrc = consts.tile([128, 16], F32, tag="wsrc")
        nc.vector.memset(wsrc[:], 0.0)
        wdst = consts.tile([128, 16], F16, tag="wdst")
        nc.scalar.activation(wdst[:], wsrc[:],
                             mybir.ActivationFunctionType.Exp, scale=SCALE)
        wmov = consts.tile([128, 512], F16, tag="wmov")
        nc.vector.memset(wmov[:], 0.0)
        warm_ps = outp_pool.tile([D, QC], F32, tag="outp", name="warm_ps")
        for _ in range(16):
            nc.tensor.matmul(warm_ps[:], wmov[:, 0:128], wmov[:],
                             start=True, stop=True)

        # ---- per-head staging ----
        def load_head(h):
            qt_sb = qkv_pool.tile([D, S], F16, tag="qt")
            nc.sync.dma_start(out=qt_sb[:], in_=qt_d[h])
            kt_sb = qkv_pool.tile([D, S], F16, tag="kt")
            nc.sync.dma_start(out=kt_sb[:], in_=kt_d[h])
            v_sb = qkv_pool.tile([128, NKT, D], F16, tag="v")
            nc.sync.dma_start(out=v_sb[:], in_=v_d[h])
            return qt_sb, kt_sb, v_sb

        heads_sb = [load_head(0), None]
        jobs = {}  # (h, qc) -> dict(out_ps, acc)

        def emit_pv_den(group_chunks, pt):
            # split the group by (h, qc) job; emit PV matmuls + den adds
            subs = []
            for i, (h, qc, kt) in enumerate(group_chunks):
                if subs and subs[-1][0] == (h, qc):
                    subs[-1][1].append((i, kt))
                else:
                    subs.append(((h, qc), [(i, kt)]))
            for (h, qc), items in subs:
                job = jobs.get((h, qc))
                if job is None:
                    out_ps = outp_pool.tile([D, QC], F32, tag="outp",
                                            name="out_ps")
                    acc = acc_pool.tile([128, GA, QC], F16, tag="acc",
                                        name="acc")
                    nc.vector.memset(acc[:], 0.0)
                    job = {"out_ps": out_ps, "acc": acc}
                    jobs[(h, qc)] = job
                v_sb = heads_sb[h][2]
                for i, kt in items:
                    nc.tensor.matmul(job["out_ps"][:], v_sb[:, kt, :],
                                     pt[:, i, :],
                                     start=(kt == 0), stop=(kt == NKT - 1))
                i0 = items[0][0]
                n = len(items)
                nc.vector.tensor_add(job["acc"][:, 0:n, :],
                                     job["acc"][:, 0:n, :],
                                     pt[:, i0:i0 + n, :])
                if items[-1][1] == NKT - 1:
                    osb = osb_pool.tile([D, QC], F32, tag="osb")
                    nc.vector.tensor_copy(osb[:], job["out_ps"][:])
                    nc.sync.dma_start(out=out_d[h, qc], in_=osb[:])
                    nc.sync.dma_start(out=acc_d[h, qc], in_=job["acc"][:])
                    del jobs[(h, qc)]

        prev = None
        for gi, (use_a, group_chunks) in enumerate(groups):
            h0 = group_chunks[0][0]
            # prefetch head 1 two jobs ahead of first use
            if heads_sb[1] is None and group_chunks[0][1] >= NQC - 2 \
                    and h0 == 0:
                heads_sb[1] = load_head(1)
            n = len(group_chunks)
            pool = stA_pool if use_a else stB_pool
            st = pool.tile([128, GA if use_a else GB, QC], F32,
                           tag="stA" if use_a else "stB", name="st")
            for i, (h, qc, kt) in enumerate(group_chunks):
                qt_sb, kt_sb, _ = heads_sb[h]
                nc.tensor.matmul(
                    st[:, i, :],
                    kt_sb[:, kt * KT:(kt + 1) * KT],
                    qt_sb[:, qc * QC:(qc + 1) * QC],
                    start=True, stop=True)
            pt = pt_pool.tile([128, GA, QC], F16, tag="pt", name="pt")
            nc.scalar.activation(pt[:, 0:n, :], st[:, 0:n, :],
                                 mybir.ActivationFunctionType.Exp,
                                 scale=SCALE)
            if prev is not None:
                emit_pv_den(*prev)
            prev = (group_chunks, pt)
        emit_pv_den(*prev)

    nc.compile()
    return nc


def _install_ntff_hook():
    """Provide antenv.axon_hooks (absent in this image) so that
    run_bass_kernel_spmd(trace=True) can capture NTFF profiles via the
    axon .so."""
    try:
        from antenv.axon_hooks import get_axon_ntff_profile_hook  # noqa: F401
        return
    except ImportError:
        pass
    import contextlib
    import ctypes
    import types

    so_path = "/opt/axon/libaxon_pjrt.so"
    lib = ctypes.CDLL(so_path)
    if not hasattr(lib, "axon_start_nrt_profile"):
        return
    lib.axon_start_nrt_profile.argtypes = [
        ctypes.POINTER(ctypes.c_int64), ctypes.c_size_t]
    lib.axon_start_nrt_profile.restype = ctypes.c_int64
    lib.axon_stop_nrt_profile.argtypes = [ctypes.c_char_p]
    lib.axon_stop_nrt_profile.restype = ctypes.c_int64

    @contextlib.contextmanager
    def _hook(output_dir, device_ids):
        import jax
        jax.devices()
        if device_ids:
            ids = (ctypes.c_int64 * len(device_ids))(*device_ids)
            rc = lib.axon_start_nrt_profile(ids, len(device_ids))
        else:
            rc = lib.axon_start_nrt_profile(None, 0)
        if rc != 0:
            raise RuntimeError(f"axon_start_nrt_profile rc={rc}")
        try:
            yield
        finally:
            n = lib.axon_stop_nrt_profile(str(output_dir).encode())
            print(f"ntff profile: {n} file(s) written to {output_dir}")

    mod = types.ModuleType("antenv.axon_hooks")
    mod.get_axon_ntff_profile_hook = lambda: _hook
    mod.set_axon_ntff_profile_hook = lambda h: None
    import antenv
    sys.modules["antenv.axon_hooks"] = mod
    antenv.axon_hooks = mod


_CACHE = {}


def _get_program():
    key = "main"
    if key not in _CACHE:
        _CACHE[key] = build_program()
    return _CACHE[key]


def kernel(query, key, value, trace=False, **trace_kwargs):
    assert query.shape == (1, S, H, D)
    nc = _get_program()

    q = np.asarray(query, dtype=np.float32)[0]   # [S, H, D]
    k = np.asarray(key, dtype=np.float32)[0]
    v = np.asarray(value, dtype=np.float32)[0]

    in_maps = []
    for c in range(N_CORES):
        hs = slice(c * HPC, (c + 1) * HPC)
        # [S, h, D] -> [h, D, S] fp16
        qt = np.ascontiguousarray(
            q[:, hs, :].transpose(1, 2, 0)).astype(np.float16)
        kt = np.ascontiguousarray(
            k[:, hs, :].transpose(1, 2, 0)).astype(np.float16)
        # [S, h, D] -> [h, 128, NKT, D] fp16  (s = kt*128 + p)
        vv = np.ascontiguousarray(
            v[:, hs, :].transpose(1, 0, 2).reshape(HPC, NKT, 128, D)
            .transpose(0, 2, 1, 3)).astype(np.float16)
        in_maps.append({"qt": qt, "kt": kt, "v": vv})

    if trace:
        _install_ntff_hook()
    res = run_bass_kernel_spmd(nc, in_maps, core_ids=list(range(N_CORES)),
                               trace=trace, **trace_kwargs)

    out = np.empty((1, S, H, D), dtype=np.float32)
    for c in range(N_CORES):
        o = res.results[c]["out"]    # [HPC, NQC, D, QC] unnormalized f32
        a = res.results[c]["acc"]    # [HPC, NQC, 128, GA, QC] f16
        den = a.astype(np.float32).sum(axis=(2, 3))  # [HPC, NQC, QC]
        on = o / den[:, :, None, :]                  # [HPC, NQC, D, QC]
        # -> [HPC, S, D]
        on = on.transpose(0, 1, 3, 2).reshape(HPC, S, D)
        for i in range(HPC):
            out[0, :, c * HPC + i, :] = on[i]
    if trace:
        kernel.last_results = res
    return out


# revision 5
# speedup vs baseline: 1.4884x; 1.1676x over previous
"""Trainium2 Bass kernel: full (non-causal) softmax attention.

Input:  query/key/value [1, 4096, 16, 128] f32 (B, S, H, D).
Output: [1, 4096, 16, 128] f32 = softmax(Q K^T / sqrt(D)) V per head.

Sharding: 16 heads over 8 cores -> 2 heads per core, no collectives.
Host pre-transposes Q,K per head to [D, S] in fp16; the device returns
the UN-normalized attention output [D, 512] per (head, q-chunk) plus
fp16 partial denominator accumulators [128, 4, 512]; the host reduces
the accumulators (sum over 128 k-lanes x 4 slots) and does the final
divide (cheap numpy).

Device pipeline (ACT-exp is the throughput floor, ~246us/core):
  - global stream of 512 score chunks (2 heads x 8 q-chunks x 32 kt),
    grouped into alternating 4-bank / 3-bank PSUM super-tiles
  - per group: PE writes scores (fp16 matmuls, N=512/bank), one big
    ACT exp (N=2048/1536, fp32 psum -> fp16 sbuf), then PE PV matmuls
    accumulate into a single out bank; DVE accumulates the softmax
    denominator with fp16 2x-mode adds into per-job accumulators.
  - software-pipelined by one group so PE never waits on ACT.
"""

import sys
from contextlib import ExitStack

import numpy as np

sys.path.insert(0, "/opt/trn_rl_repo")

import concourse.bacc as bacc
import concourse.bass as bass
import concourse.tile as tile
from concourse import mybir
from concourse.bass_utils import run_bass_kernel_spmd

N_CORES = 8
S = 4096
H = 16
D = 128
HPC = H // N_CORES   # heads per core = 2
QC = 512             # queries per job (one psum bank of fp32)
NQC = S // QC        # 8 q-chunks per head
KT = 128             # keys per score chunk (psum partition dim)
NKT = S // KT        # 32 key chunks
SCALE = float(D) ** -0.5
GA = 4               # chunks per group in buffer A (4 psum banks)
GB = 3               # chunks per group in buffer B (3 psum banks)

F32 = mybir.dt.float32
F16 = mybir.dt.float16


def _make_groups():
    """Global chunk stream split into alternating A/B groups."""
    chunks = [(h, qc, kt)
              for h in range(HPC) for qc in range(NQC) for kt in range(NKT)]
    groups = []
    i = 0
    use_a = True
    while i < len(chunks):
        n = min(GA if use_a else GB, len(chunks) - i)
        groups.append((use_a, chunks[i:i + n]))
        i += n
        use_a = not use_a
    return groups


def build_program():
    nc = bacc.Bacc("TRN2", target_bir_lowering=False, debug=False,
                   num_devices=N_CORES)

    qt_d = nc.dram_tensor("qt", [HPC, D, S], F16, kind="ExternalInput")
    kt_d = nc.dram_tensor("kt", [HPC, D, S], F16, kind="ExternalInput")
    v_d = nc.dram_tensor("v", [HPC, 128, NKT, D], F16, kind="ExternalInput")
    out_d = nc.dram_tensor("out", [HPC, NQC, D, QC], F32,
                           kind="ExternalOutput")
    acc_d = nc.dram_tensor("acc", [HPC, NQC, 128, GA, QC], F16,
                           kind="ExternalOutput")

    groups = _make_groups()

    with tile.TileContext(nc) as tc, ExitStack() as ctx:
        consts = ctx.enter_context(tc.tile_pool(name="consts", bufs=1))
        qkv_pool = ctx.enter_context(tc.tile_pool(name="qkv", bufs=2))
        pt_pool = ctx.enter_context(tc.tile_pool(name="pt", bufs=5))
        acc_pool = ctx.enter_context(tc.tile_pool(name="acc", bufs=2))
        osb_pool = ctx.enter_context(tc.tile_pool(name="osb", bufs=3))
        stA_pool = ctx.enter_context(
            tc.tile_pool(name="stA", bufs=1, space="PSUM"))
        stB_pool = ctx.enter_context(
            tc.tile_pool(name="stB", bufs=1, space="PSUM"))
        outp_pool = ctx.enter_context(
            tc.tile_pool(name="outp", bufs=1, space="PSUM"))

        # ---- warmup: ACT table load + PE HAM ramp while DMAs run ----
        wsrc = consts.tile([128, 16], F32, tag="wsrc")
        nc.vector.memset(wsrc[:], 0.0)
        wdst = consts.tile([128, 16], F16, tag="wdst")
        nc.scalar.activation(wdst[:], wsrc[:],
                             mybir.ActivationFunctionType.Exp, scale=SCALE)
        wmov = consts.tile([128, 512], F16, tag="wmov")
        nc.vector.memset(wmov[:], 0.0)
        warm_ps = outp_pool.tile([D, QC], F32, tag="outp", name="warm_ps")
        for _ in range(16):
            nc.tensor.matmul(warm_ps[:], wmov[:, 0:128], wmov[:],
                             start=True, stop=True)

        # ---- per-head staging ----
        def load_head(h, chunk_first=False):
            qt_sb = qkv_pool.tile([D, S], F16, tag="qt")
            kt_sb = qkv_pool.tile([D, S], F16, tag="kt")
            v_sb = qkv_pool.tile([128, NKT, D], F16, tag="v")
            if chunk_first:
                # split so the first group's operands land fast
                nc.sync.dma_start(out=kt_sb[:, 0:QC], in_=kt_d[h][:, 0:QC])
                nc.sync.dma_start(out=qt_sb[:, 0:QC], in_=qt_d[h][:, 0:QC])
                nc.sync.dma_start(out=kt_sb[:, QC:], in_=kt_d[h][:, QC:])
                nc.sync.dma_start(out=qt_sb[:, QC:], in_=qt_d[h][:, QC:])
            else:
                nc.sync.dma_start(out=qt_sb[:], in_=qt_d[h])
                nc.sync.dma_start(out=kt_sb[:], in_=kt_d[h])
            nc.sync.dma_start(out=v_sb[:], in_=v_d[h])
            return qt_sb, kt_sb, v_sb

        heads_sb = [load_head(0, chunk_first=True), None]
        jobs = {}  # (h, qc) -> dict(out_ps, acc)

        def emit_pv_den(group_chunks, pt):
            # split the group by (h, qc) job; emit PV matmuls + den adds
            subs = []
            for i, (h, qc, kt) in enumerate(group_chunks):
                if subs and subs[-1][0] == (h, qc):
                    subs[-1][1].append((i, kt))
                else:
                    subs.append(((h, qc), [(i, kt)]))
            for (h, qc), items in subs:
                i0 = items[0][0]
                n = len(items)
                job = jobs.get((h, qc))
                fresh = job is None
                if fresh:
                    out_ps = outp_pool.tile([D, QC], F32, tag="outp",
                                            name="out_ps")
                    acc = acc_pool.tile([128, GA, QC], F16, tag="acc",
                                        name="acc")
                    job = {"out_ps": out_ps, "acc": acc}
                    jobs[(h, qc)] = job
                v_sb = heads_sb[h][2]
                for i, kt in items:
                    nc.tensor.matmul(job["out_ps"][:], v_sb[:, kt, :],
                                     pt[:, i, :],
                                     start=(kt == 0), stop=(kt == NKT - 1))
                if fresh:
                    # first touch: copy instead of memset+add
                    nc.vector.tensor_copy(job["acc"][:, 0:n, :],
                                          pt[:, i0:i0 + n, :])
                    if n < GA:
                        nc.vector.memset(job["acc"][:, n:GA, :], 0.0)
                else:
                    nc.vector.tensor_add(job["acc"][:, 0:n, :],
                                         job["acc"][:, 0:n, :],
                                         pt[:, i0:i0 + n, :])
                if items[-1][1] == NKT - 1:
                    osb = osb_pool.tile([D, QC], F32, tag="osb")
                    nc.vector.tensor_copy(osb[:], job["out_ps"][:])
                    nc.sync.dma_start(out=out_d[h, qc], in_=osb[:])
                    nc.sync.dma_start(out=acc_d[h, qc], in_=job["acc"][:])
                    del jobs[(h, qc)]

        # PV/den deferred by TWO groups: keeps exp(g) -> scores(g+2) off
        # the PV path so ACT runs back-to-back.
        pending = []
        for gi, (use_a, group_chunks) in enumerate(groups):
            h0 = group_chunks[0][0]
            # prefetch head 1 two jobs ahead of first use
            if heads_sb[1] is None and group_chunks[0][1] >= NQC - 2 \
                    and h0 == 0:
                heads_sb[1] = load_head(1)
            n = len(group_chunks)
            pool = stA_pool if use_a else stB_pool
            st = pool.tile([128, GA if use_a else GB, QC], F32,
                           tag="stA" if use_a else "stB", name="st")
            for i, (h, qc, kt) in enumerate(group_chunks):
                qt_sb, kt_sb, _ = heads_sb[h]
                nc.tensor.matmul(
                    st[:, i, :],
                    kt_sb[:, kt * KT:(kt + 1) * KT],
                    qt_sb[:, qc * QC:(qc + 1) * QC],
                    start=True, stop=True)
            pt = pt_pool.tile([128, GA, QC], F16, tag="pt", name="pt")
            nc.scalar.activation(pt[:, 0:n, :], st[:, 0:n, :],
                                 mybir.ActivationFunctionType.Exp,
                                 scale=SCALE)
            pending.append((group_chunks, pt))
            if len(pending) > 2:
                emit_pv_den(*pending.pop(0))
        while pending:
            emit_pv_den(*pending.pop(0))

    nc.compile()
    return nc


def _install_ntff_hook():
    """Provide antenv.axon_hooks (absent in this image) so that
    run_bass_kernel_spmd(trace=True) can capture NTFF profiles via the
    axon .so."""
    try:
        from antenv.axon_hooks import get_axon_ntff_profile_hook  # noqa: F401
        return
    except ImportError:
        pass
    import contextlib
    import ctypes
    import types

    so_path = "/opt/axon/libaxon_pjrt.so"
    lib = ctypes.CDLL(so_path)
    if not hasattr(lib, "axon_start_nrt_profile"):
        return
    lib.axon_start_nrt_profile.argtypes = [
        ctypes.POINTER(ctypes.c_int64), ctypes.c_size_t]
    lib.axon_start_nrt_profile.restype = ctypes.c_int64
    lib.axon_stop_nrt_profile.argtypes = [ctypes.c_char_p]
    lib.axon_stop_nrt_profile.restype = ctypes.c_int64

    @contextlib.contextmanager
    def _hook(output_dir, device_ids):
        import jax
        jax.devices()
        if device_ids:
            ids = (ctypes.c_int64 * len(device_ids))(*device_ids)
            rc = lib.axon_start_nrt_profile(ids, len(device_ids))
        else:
            rc = lib.axon_start_nrt_profile(None, 0)
        if rc != 0:
            raise RuntimeError(f"axon_start_nrt_profile rc={rc}")
        try:
            yield
        finally:
            n = lib.axon_stop_nrt_profile(str(output_dir).encode())
            print(f"ntff profile: {n} file(s) written to {output_dir}")

    mod = types.ModuleType("antenv.axon_hooks")
    mod.get_axon_ntff_profile_hook = lambda: _hook
    mod.set_axon_ntff_profile_hook = lambda h: None
    import antenv
    sys.modules["antenv.axon_hooks"] = mod
    antenv.axon_hooks = mod


_CACHE = {}


def _get_program():
    key = "main"
    if key not in _CACHE:
        _CACHE[key] = build_program()
    return _CACHE[key]


def kernel(query, key, value, trace=False, **trace_kwargs):
    assert query.shape == (1, S, H, D)
    nc = _get_program()

    q = np.asarray(query, dtype=np.float32)[0]   # [S, H, D]
    k = np.asarray(key, dtype=np.float32)[0]
    v = np.asarray(value, dtype=np.float32)[0]

    in_maps = []
    for c in range(N_CORES):
        hs = slice(c * HPC, (c + 1) * HPC)
        # [S, h, D] -> [h, D, S] fp16
        qt = np.ascontiguousarray(
            q[:, hs, :].transpose(1, 2, 0)).astype(np.float16)
        kt = np.ascontiguousarray(
            k[:, hs, :].transpose(1, 2, 0)).astype(np.float16)
        # [S, h, D] -> [h, 128, NKT, D] fp16  (s = kt*128 + p)
        vv = np.ascontiguousarray(
            v[:, hs, :].transpose(1, 0, 2).reshape(HPC, NKT, 128, D)
            .transpose(0, 2, 1, 3)).astype(np.float16)
        in_maps.append({"qt": qt, "kt": kt, "v": vv})

    if trace:
        _install_ntff_hook()
    res = run_bass_kernel_spmd(nc, in_maps, core_ids=list(range(N_CORES)),
                               trace=trace, **trace_kwargs)

    out = np.empty((1, S, H, D), dtype=np.float32)
    for c in range(N_CORES):
        o = res.results[c]["out"]    # [HPC, NQC, D, QC] unnormalized f32
        a = res.results[c]["acc"]    # [HPC, NQC, 128, GA, QC] f16
        den = a.astype(np.float32).sum(axis=(2, 3))  # [HPC, NQC, QC]
        on = o / den[:, :, None, :]                  # [HPC, NQC, D, QC]
        # -> [HPC, S, D]
        on = on.transpose(0, 1, 3, 2).reshape(HPC, S, D)
        for i in range(HPC):
            out[0, :, c * HPC + i, :] = on[i]
    if trace:
        kernel.last_results = res
    return out


# revision 10
# speedup vs baseline: 1.4998x; 1.0077x over previous
"""Trainium2 Bass kernel: full (non-causal) softmax attention.

Input:  query/key/value [1, 4096, 16, 128] f32 (B, S, H, D).
Output: [1, 4096, 16, 128] f32 = softmax(Q K^T / sqrt(D)) V per head.

Sharding: 16 heads over 8 cores -> 2 heads per core, no collectives.
Host pre-transposes Q,K per head to [D, S] in fp16; the device returns
the UN-normalized attention output [D, 512] per (head, q-chunk) plus
fp16 partial denominator accumulators [128, 4, 512]; the host reduces
the accumulators (sum over 128 k-lanes x 4 slots) and does the final
divide (cheap numpy).

Device pipeline (ACT-exp is the throughput floor, ~246us/core):
  - global stream of 512 score chunks (2 heads x 8 q-chunks x 32 kt),
    grouped into alternating 4-bank / 3-bank PSUM super-tiles
  - per group: PE writes scores (fp16 matmuls, N=512/bank), one big
    ACT exp (N=2048/1536, fp32 psum -> fp16 sbuf), then PE PV matmuls
    accumulate into a single out bank; DVE accumulates the softmax
    denominator with fp16 2x-mode adds into per-job accumulators.
  - software-pipelined by one group so PE never waits on ACT.
"""

import sys
from contextlib import ExitStack

import numpy as np

sys.path.insert(0, "/opt/trn_rl_repo")

import concourse.bacc as bacc
import concourse.bass as bass
import concourse.tile as tile
from concourse import mybir
from concourse.bass_utils import run_bass_kernel_spmd

N_CORES = 8
S = 4096
H = 16
D = 128
HPC = H // N_CORES   # heads per core = 2
QC = 512             # queries per job (one psum bank of fp32)
NQC = S // QC        # 8 q-chunks per head
KT = 128             # keys per score chunk (psum partition dim)
NKT = S // KT        # 32 key chunks
SCALE = float(D) ** -0.5
GA = 4               # chunks per group in buffer A (4 psum banks)
GB = 3               # chunks per group in buffer B (3 psum banks)

F32 = mybir.dt.float32
F16 = mybir.dt.float16


def _make_groups():
    """Global chunk stream split into alternating A/B groups."""
    chunks = [(h, qc, kt)
              for h in range(HPC) for qc in range(NQC) for kt in range(NKT)]
    groups = []
    i = 0
    use_a = True
    while i < len(chunks):
        n = min(GA if use_a else GB, len(chunks) - i)
        groups.append((use_a, chunks[i:i + n]))
        i += n
        use_a = not use_a
    return groups


def build_program():
    nc = bacc.Bacc("TRN2", target_bir_lowering=False, debug=False,
                   num_devices=N_CORES)

    qt_d = nc.dram_tensor("qt", [HPC, D, S], F16, kind="ExternalInput")
    kt_d = nc.dram_tensor("kt", [HPC, D, S], F16, kind="ExternalInput")
    v_d = nc.dram_tensor("v", [HPC, 128, NKT, D], F16, kind="ExternalInput")
    out_d = nc.dram_tensor("out", [HPC, NQC, D, QC], F16,
                           kind="ExternalOutput")
    acc_d = nc.dram_tensor("acc", [HPC, NQC, 128, 2, QC], F16,
                           kind="ExternalOutput")

    groups = _make_groups()

    with tile.TileContext(nc) as tc, ExitStack() as ctx:
        consts = ctx.enter_context(tc.tile_pool(name="consts", bufs=1))
        qkv_pool = ctx.enter_context(tc.tile_pool(name="qkv", bufs=2))
        pt_pool = ctx.enter_context(tc.tile_pool(name="pt", bufs=5))
        acc_pool = ctx.enter_context(tc.tile_pool(name="acc", bufs=2))
        osb_pool = ctx.enter_context(tc.tile_pool(name="osb", bufs=3))
        stA_pool = ctx.enter_context(
            tc.tile_pool(name="stA", bufs=1, space="PSUM"))
        stB_pool = ctx.enter_context(
            tc.tile_pool(name="stB", bufs=1, space="PSUM"))
        outp_pool = ctx.enter_context(
            tc.tile_pool(name="outp", bufs=1, space="PSUM"))

        # ---- warmup: ACT table load + PE HAM ramp while DMAs run ----
        wsrc = consts.tile([128, 16], F32, tag="wsrc")
        nc.vector.memset(wsrc[:], 0.0)
        wdst = consts.tile([128, 16], F16, tag="wdst")
        nc.scalar.activation(wdst[:], wsrc[:],
                             mybir.ActivationFunctionType.Exp, scale=SCALE)
        wmov = consts.tile([128, 512], F16, tag="wmov")
        nc.vector.memset(wmov[:], 0.0)
        warm_ps = outp_pool.tile([D, QC], F32, tag="outp", name="warm_ps")
        for _ in range(8):
            nc.tensor.matmul(warm_ps[:], wmov[:, 0:128], wmov[:],
                             start=True, stop=True)

        # ---- per-head staging ----
        def load_head(h, chunk_first=False):
            qt_sb = qkv_pool.tile([D, S], F16, tag="qt")
            kt_sb = qkv_pool.tile([D, S], F16, tag="kt")
            v_sb = qkv_pool.tile([128, NKT, D], F16, tag="v")
            if chunk_first:
                # split so the first groups' operands land fast
                nc.sync.dma_start(out=kt_sb[:, 0:QC], in_=kt_d[h][:, 0:QC])
                nc.sync.dma_start(out=qt_sb[:, 0:QC], in_=qt_d[h][:, 0:QC])
                nc.sync.dma_start(out=v_sb[:, 0:8, :], in_=v_d[h][:, 0:8, :])
                nc.sync.dma_start(out=kt_sb[:, QC:], in_=kt_d[h][:, QC:])
                nc.sync.dma_start(out=qt_sb[:, QC:], in_=qt_d[h][:, QC:])
                nc.sync.dma_start(out=v_sb[:, 8:, :], in_=v_d[h][:, 8:, :])
            else:
                nc.sync.dma_start(out=qt_sb[:], in_=qt_d[h])
                nc.sync.dma_start(out=kt_sb[:], in_=kt_d[h])
                nc.sync.dma_start(out=v_sb[:], in_=v_d[h])
            return qt_sb, kt_sb, v_sb

        heads_sb = [load_head(0, chunk_first=True), None]
        jobs = {}  # (h, qc) -> dict(out_ps, acc)

        def emit_pv_den(group_chunks, pt):
            # split the group by (h, qc) job; emit PV matmuls + den adds
            subs = []
            for i, (h, qc, kt) in enumerate(group_chunks):
                if subs and subs[-1][0] == (h, qc):
                    subs[-1][1].append((i, kt))
                else:
                    subs.append(((h, qc), [(i, kt)]))
            for (h, qc), items in subs:
                i0 = items[0][0]
                n = len(items)
                job = jobs.get((h, qc))
                fresh = job is None
                if fresh:
                    out_ps = outp_pool.tile([D, QC], F32, tag="outp",
                                            name="out_ps")
                    acc = acc_pool.tile([128, GA, QC], F16, tag="acc",
                                        name="acc")
                    job = {"out_ps": out_ps, "acc": acc}
                    jobs[(h, qc)] = job
                v_sb = heads_sb[h][2]
                for i, kt in items:
                    nc.tensor.matmul(job["out_ps"][:], v_sb[:, kt, :],
                                     pt[:, i, :],
                                     start=(kt == 0), stop=(kt == NKT - 1))
                if fresh:
                    # first touch: copy instead of memset+add
                    nc.vector.tensor_copy(job["acc"][:, 0:n, :],
                                          pt[:, i0:i0 + n, :])
                    if n < GA:
                        nc.vector.memset(job["acc"][:, n:GA, :], 0.0)
                else:
                    nc.vector.tensor_add(job["acc"][:, 0:n, :],
                                         job["acc"][:, 0:n, :],
                                         pt[:, i0:i0 + n, :])
                if items[-1][1] == NKT - 1:
                    osb = osb_pool.tile([D, QC], F16, tag="osb")
                    nc.vector.tensor_copy(osb[:], job["out_ps"][:])
                    nc.sync.dma_start(out=out_d[h, qc], in_=osb[:])
                    # fold 4 den slots -> 2 to halve the DMA
                    nc.vector.tensor_add(job["acc"][:, 0:2, :],
                                         job["acc"][:, 0:2, :],
                                         job["acc"][:, 2:GA, :])
                    nc.sync.dma_start(out=acc_d[h, qc],
                                      in_=job["acc"][:, 0:2, :])
                    del jobs[(h, qc)]

        # PV/den deferred by TWO groups: keeps exp(g) -> scores(g+2) off
        # the PV path so ACT runs back-to-back.
        pending = []
        for gi, (use_a, group_chunks) in enumerate(groups):
            h0 = group_chunks[0][0]
            # prefetch head 1 two jobs ahead of first use
            if heads_sb[1] is None and group_chunks[0][1] >= NQC - 2 \
                    and h0 == 0:
                heads_sb[1] = load_head(1)
            n = len(group_chunks)
            pool = stA_pool if use_a else stB_pool
            st = pool.tile([128, GA if use_a else GB, QC], F32,
                           tag="stA" if use_a else "stB", name="st")
            for i, (h, qc, kt) in enumerate(group_chunks):
                qt_sb, kt_sb, _ = heads_sb[h]
                nc.tensor.matmul(
                    st[:, i, :],
                    kt_sb[:, kt * KT:(kt + 1) * KT],
                    qt_sb[:, qc * QC:(qc + 1) * QC],
                    start=True, stop=True)
            pt = pt_pool.tile([128, GA, QC], F16, tag="pt", name="pt")
            nc.scalar.activation(pt[:, 0:n, :], st[:, 0:n, :],
                                 mybir.ActivationFunctionType.Exp,
                                 scale=SCALE)
            pending.append((group_chunks, pt))
            if len(pending) > 2:
                emit_pv_den(*pending.pop(0))
        while pending:
            emit_pv_den(*pending.pop(0))

    nc.compile()
    return nc


def _install_ntff_hook():
    """Provide antenv.axon_hooks (absent in this image) so that
    run_bass_kernel_spmd(trace=True) can capture NTFF profiles via the
    axon .so."""
    try:
        from antenv.axon_hooks import get_axon_ntff_profile_hook  # noqa: F401
        return
    except ImportError:
        pass
    import contextlib
    import ctypes
    import types

    so_path = "/opt/axon/libaxon_pjrt.so"
    lib = ctypes.CDLL(so_path)
    if not hasattr(lib, "axon_start_nrt_profile"):
        return
    lib.axon_start_nrt_profile.argtypes = [
        ctypes.POINTER(ctypes.c_int64), ctypes.c_size_t]
    lib.axon_start_nrt_profile.restype = ctypes.c_int64
    lib.axon_stop_nrt_profile.argtypes = [ctypes.c_char_p]
    lib.axon_stop_nrt_profile.restype = ctypes.c_int64

    @contextlib.contextmanager
    def _hook(output_dir, device_ids):
        import jax
        jax.devices()
        if device_ids:
            ids = (ctypes.c_int64 * len(device_ids))(*device_ids)
            rc = lib.axon_start_nrt_profile(ids, len(device_ids))
        else:
            rc = lib.axon_start_nrt_profile(None, 0)
        if rc != 0:
            raise RuntimeError(f"axon_start_nrt_profile rc={rc}")
        try:
            yield
        finally:
            n = lib.axon_stop_nrt_profile(str(output_dir).encode())
            print(f"ntff profile: {n} file(s) written to {output_dir}")

    mod = types.ModuleType("antenv.axon_hooks")
    mod.get_axon_ntff_profile_hook = lambda: _hook
    mod.set_axon_ntff_profile_hook = lambda h: None
    import antenv
    sys.modules["antenv.axon_hooks"] = mod
    antenv.axon_hooks = mod


_CACHE = {}


def _get_program():
    key = "main"
    if key not in _CACHE:
        _CACHE[key] = build_program()
    return _CACHE[key]


def kernel(query, key, value, trace=False, **trace_kwargs):
    assert query.shape == (1, S, H, D)
    nc = _get_program()

    q = np.asarray(query, dtype=np.float32)[0]   # [S, H, D]
    k = np.asarray(key, dtype=np.float32)[0]
    v = np.asarray(value, dtype=np.float32)[0]

    in_maps = []
    for c in range(N_CORES):
        hs = slice(c * HPC, (c + 1) * HPC)
        # [S, h, D] -> [h, D, S] fp16
        qt = np.ascontiguousarray(
            q[:, hs, :].transpose(1, 2, 0)).astype(np.float16)
        kt = np.ascontiguousarray(
            k[:, hs, :].transpose(1, 2, 0)).astype(np.float16)
        # [S, h, D] -> [h, 128, NKT, D] fp16  (s = kt*128 + p)
        vv = np.ascontiguousarray(
            v[:, hs, :].transpose(1, 0, 2).reshape(HPC, NKT, 128, D)
            .transpose(0, 2, 1, 3)).astype(np.float16)
        in_maps.append({"qt": qt, "kt": kt, "v": vv})

    if trace:
        _install_ntff_hook()
    res = run_bass_kernel_spmd(nc, in_maps, core_ids=list(range(N_CORES)),
                               trace=trace, **trace_kwargs)

    out = np.empty((1, S, H, D), dtype=np.float32)
    for c in range(N_CORES):
        o = res.results[c]["out"].astype(np.float32)  # [HPC, NQC, D, QC]
        a = res.results[c]["acc"]    # [HPC, NQC, 128, 2, QC] f16
        den = a.astype(np.float32).sum(axis=(2, 3))  # [HPC, NQC, QC]
        on = o / den[:, :, None, :]                  # [HPC, NQC, D, QC]
        # -> [HPC, S, D]
        on = on.transpose(0, 1, 3, 2).reshape(HPC, S, D)
        for i in range(HPC):
            out[0, :, c * HPC + i, :] = on[i]
    if trace:
        kernel.last_results = res
    return out


# revision 12
# speedup vs baseline: 1.5108x; 1.0073x over previous
"""Trainium2 Bass kernel: full (non-causal) softmax attention.

Input:  query/key/value [1, 4096, 16, 128] f32 (B, S, H, D).
Output: [1, 4096, 16, 128] f32 = softmax(Q K^T / sqrt(D)) V per head.

Sharding: 16 heads over 8 cores -> 2 heads per core, no collectives.
Host pre-transposes Q,K per head to [D, S] in fp16; the device returns
the UN-normalized attention output [D, 512] per (head, q-chunk) plus
fp16 partial denominator accumulators [128, 4, 512]; the host reduces
the accumulators (sum over 128 k-lanes x 4 slots) and does the final
divide (cheap numpy).

Device pipeline (ACT-exp is the throughput floor, ~246us/core):
  - global stream of 512 score chunks (2 heads x 8 q-chunks x 32 kt),
    grouped into alternating 4-bank / 3-bank PSUM super-tiles
  - per group: PE writes scores (fp16 matmuls, N=512/bank), one big
    ACT exp (N=2048/1536, fp32 psum -> fp16 sbuf), then PE PV matmuls
    accumulate into a single out bank; DVE accumulates the softmax
    denominator with fp16 2x-mode adds into per-job accumulators.
  - software-pipelined by one group so PE never waits on ACT.
"""

import sys
from contextlib import ExitStack

import numpy as np

sys.path.insert(0, "/opt/trn_rl_repo")

import concourse.bacc as bacc
import concourse.bass as bass
import concourse.tile as tile
from concourse import mybir
from concourse.bass_utils import run_bass_kernel_spmd

N_CORES = 8
S = 4096
H = 16
D = 128
HPC = H // N_CORES   # heads per core = 2
QC = 512             # queries per job (one psum bank of fp32)
NQC = S // QC        # 8 q-chunks per head
KT = 128             # keys per score chunk (psum partition dim)
NKT = S // KT        # 32 key chunks
SCALE = float(D) ** -0.5
GA = 4               # chunks per group in buffer A (4 psum banks)
GB = 3               # chunks per group in buffer B (3 psum banks)

F32 = mybir.dt.float32
F16 = mybir.dt.float16


def _make_groups():
    """Global chunk stream split into alternating A/B groups.

    The remainder (size-1) group leads the stream: a 1-chunk group gets
    the first exp onto ACT as soon as its table load finishes.
    """
    chunks = [(h, qc, kt)
              for h in range(HPC) for qc in range(NQC) for kt in range(NKT)]
    sizes = [1]
    use_a = False
    left = len(chunks) - 1
    while left:
        n = min(GA if use_a else GB, left)
        sizes.append(n)
        left -= n
        use_a = not use_a
    groups = []
    i = 0
    for gi, n in enumerate(sizes):
        groups.append((gi % 2 == 0, chunks[i:i + n]))
        i += n
    return groups


def build_program():
    nc = bacc.Bacc("TRN2", target_bir_lowering=False, debug=False,
                   num_devices=N_CORES)

    qt_d = nc.dram_tensor("qt", [HPC, D, S], F16, kind="ExternalInput")
    kt_d = nc.dram_tensor("kt", [HPC, D, S], F16, kind="ExternalInput")
    v_d = nc.dram_tensor("v", [HPC, 128, NKT, D], F16, kind="ExternalInput")
    out_d = nc.dram_tensor("out", [HPC, NQC, D, QC], F16,
                           kind="ExternalOutput")
    acc_d = nc.dram_tensor("acc", [HPC, NQC, 128, 2, QC], F16,
                           kind="ExternalOutput")

    groups = _make_groups()

    with tile.TileContext(nc) as tc, ExitStack() as ctx:
        consts = ctx.enter_context(tc.tile_pool(name="consts", bufs=1))
        qkv_pool = ctx.enter_context(tc.tile_pool(name="qkv", bufs=2))
        pt_pool = ctx.enter_context(tc.tile_pool(name="pt", bufs=5))
        acc_pool = ctx.enter_context(tc.tile_pool(name="acc", bufs=2))
        osb_pool = ctx.enter_context(tc.tile_pool(name="osb", bufs=3))
        stA_pool = ctx.enter_context(
            tc.tile_pool(name="stA", bufs=1, space="PSUM"))
        stB_pool = ctx.enter_context(
            tc.tile_pool(name="stB", bufs=1, space="PSUM"))
        outp_pool = ctx.enter_context(
            tc.tile_pool(name="outp", bufs=1, space="PSUM"))

        # ---- warmup: ACT table load + PE HAM ramp while DMAs run ----
        wsrc = consts.tile([128, 16], F32, tag="wsrc")
        nc.vector.memset(wsrc[:], 0.0)
        wdst = consts.tile([128, 16], F16, tag="wdst")
        nc.scalar.activation(wdst[:], wsrc[:],
                             mybir.ActivationFunctionType.Exp, scale=SCALE)


        # ---- per-head staging ----
        def load_head(h, chunk_first=False):
            qt_sb = qkv_pool.tile([D, S], F16, tag="qt")
            kt_sb = qkv_pool.tile([D, S], F16, tag="kt")
            v_sb = qkv_pool.tile([128, NKT, D], F16, tag="v")
            if chunk_first:
                # split so the first groups' operands land fast
                nc.sync.dma_start(out=kt_sb[:, 0:QC], in_=kt_d[h][:, 0:QC])
                nc.sync.dma_start(out=qt_sb[:, 0:QC], in_=qt_d[h][:, 0:QC])
                nc.sync.dma_start(out=v_sb[:, 0:8, :], in_=v_d[h][:, 0:8, :])
                nc.sync.dma_start(out=kt_sb[:, QC:], in_=kt_d[h][:, QC:])
                nc.sync.dma_start(out=qt_sb[:, QC:], in_=qt_d[h][:, QC:])
                nc.sync.dma_start(out=v_sb[:, 8:, :], in_=v_d[h][:, 8:, :])
            else:
                nc.sync.dma_start(out=qt_sb[:], in_=qt_d[h])
                nc.sync.dma_start(out=kt_sb[:], in_=kt_d[h])
                nc.sync.dma_start(out=v_sb[:], in_=v_d[h])
            return qt_sb, kt_sb, v_sb

        heads_sb = [load_head(0, chunk_first=True), None]
        jobs = {}  # (h, qc) -> dict(out_ps, acc)

        def emit_pv_den(group_chunks, pt):
            # split the group by (h, qc) job; emit PV matmuls + den adds
            subs = []
            for i, (h, qc, kt) in enumerate(group_chunks):
                if subs and subs[-1][0] == (h, qc):
                    subs[-1][1].append((i, kt))
                else:
                    subs.append(((h, qc), [(i, kt)]))
            for (h, qc), items in subs:
                i0 = items[0][0]
                n = len(items)
                job = jobs.get((h, qc))
                fresh = job is None
                if fresh:
                    out_ps = outp_pool.tile([D, QC], F32, tag="outp",
                                            name="out_ps")
                    acc = acc_pool.tile([128, GA, QC], F16, tag="acc",
                                        name="acc")
                    job = {"out_ps": out_ps, "acc": acc}
                    jobs[(h, qc)] = job
                v_sb = heads_sb[h][2]
                for i, kt in items:
                    nc.tensor.matmul(job["out_ps"][:], v_sb[:, kt, :],
                                     pt[:, i, :],
                                     start=(kt == 0), stop=(kt == NKT - 1))
                if fresh:
                    # first touch: copy instead of memset+add
                    nc.vector.tensor_copy(job["acc"][:, 0:n, :],
                                          pt[:, i0:i0 + n, :])
                    if n < GA:
                        nc.vector.memset(job["acc"][:, n:GA, :], 0.0)
                else:
                    nc.vector.tensor_add(job["acc"][:, 0:n, :],
                                         job["acc"][:, 0:n, :],
                                         pt[:, i0:i0 + n, :])
                if items[-1][1] == NKT - 1:
                    osb = osb_pool.tile([D, QC], F16, tag="osb")
                    nc.vector.tensor_copy(osb[:], job["out_ps"][:])
                    nc.sync.dma_start(out=out_d[h, qc], in_=osb[:])
                    # fold 4 den slots -> 2 to halve the DMA
                    nc.vector.tensor_add(job["acc"][:, 0:2, :],
                                         job["acc"][:, 0:2, :],
                                         job["acc"][:, 2:GA, :])
                    nc.sync.dma_start(out=acc_d[h, qc],
                                      in_=job["acc"][:, 0:2, :])
                    del jobs[(h, qc)]

        # PV/den deferred by TWO groups: keeps exp(g) -> scores(g+2) off
        # the PV path so ACT runs back-to-back.
        pending = []
        for gi, (use_a, group_chunks) in enumerate(groups):
            h0 = group_chunks[0][0]
            # prefetch head 1 two jobs ahead of first use
            if heads_sb[1] is None and group_chunks[0][1] >= NQC - 2 \
                    and h0 == 0:
                heads_sb[1] = load_head(1)
            n = len(group_chunks)
            pool = stA_pool if use_a else stB_pool
            st = pool.tile([128, GA if use_a else GB, QC], F32,
                           tag="stA" if use_a else "stB", name="st")
            for i, (h, qc, kt) in enumerate(group_chunks):
                qt_sb, kt_sb, _ = heads_sb[h]
                nc.tensor.matmul(
                    st[:, i, :],
                    kt_sb[:, kt * KT:(kt + 1) * KT],
                    qt_sb[:, qc * QC:(qc + 1) * QC],
                    start=True, stop=True)
            pt = pt_pool.tile([128, GA, QC], F16, tag="pt", name="pt")
            nc.scalar.activation(pt[:, 0:n, :], st[:, 0:n, :],
                                 mybir.ActivationFunctionType.Exp,
                                 scale=SCALE)
            pending.append((group_chunks, pt))
            if len(pending) > 2:
                emit_pv_den(*pending.pop(0))
        while pending:
            emit_pv_den(*pending.pop(0))

    nc.compile()
    return nc


def _install_ntff_hook():
    """Provide antenv.axon_hooks (absent in this image) so that
    run_bass_kernel_spmd(trace=True) can capture NTFF profiles via the
    axon .so."""
    try:
        from antenv.axon_hooks import get_axon_ntff_profile_hook  # noqa: F401
        return
    except ImportError:
        pass
    import contextlib
    import ctypes
    import types

    so_path = "/opt/axon/libaxon_pjrt.so"
    lib = ctypes.CDLL(so_path)
    if not hasattr(lib, "axon_start_nrt_profile"):
        return
    lib.axon_start_nrt_profile.argtypes = [
        ctypes.POINTER(ctypes.c_int64), ctypes.c_size_t]
    lib.axon_start_nrt_profile.restype = ctypes.c_int64
    lib.axon_stop_nrt_profile.argtypes = [ctypes.c_char_p]
    lib.axon_stop_nrt_profile.restype = ctypes.c_int64

    @contextlib.contextmanager
    def _hook(output_dir, device_ids):
        import jax
        jax.devices()
        if device_ids:
            ids = (ctypes.c_int64 * len(device_ids))(*device_ids)
            rc = lib.axon_start_nrt_profile(ids, len(device_ids))
        else:
            rc = lib.axon_start_nrt_profile(None, 0)
        if rc != 0:
            raise RuntimeError(f"axon_start_nrt_profile rc={rc}")
        try:
            yield
        finally:
            n = lib.axon_stop_nrt_profile(str(output_dir).encode())
            print(f"ntff profile: {n} file(s) written to {output_dir}")

    mod = types.ModuleType("antenv.axon_hooks")
    mod.get_axon_ntff_profile_hook = lambda: _hook
    mod.set_axon_ntff_profile_hook = lambda h: None
    import antenv
    sys.modules["antenv.axon_hooks"] = mod
    antenv.axon_hooks = mod


_CACHE = {}


def _get_program():
    key = "main"
    if key not in _CACHE:
        _CACHE[key] = build_program()
    return _CACHE[key]


def kernel(query, key, value, trace=False, **trace_kwargs):
    assert query.shape == (1, S, H, D)
    nc = _get_program()

    q = np.asarray(query, dtype=np.float32)[0]   # [S, H, D]
    k = np.asarray(key, dtype=np.float32)[0]
    v = np.asarray(value, dtype=np.float32)[0]

    in_maps = []
    for c in range(N_CORES):
        hs = slice(c * HPC, (c + 1) * HPC)
        # [S, h, D] -> [h, D, S] fp16
        qt = np.ascontiguousarray(
            q[:, hs, :].transpose(1, 2, 0)).astype(np.float16)
        kt = np.ascontiguousarray(
            k[:, hs, :].transpose(1, 2, 0)).astype(np.float16)
        # [S, h, D] -> [h, 128, NKT, D] fp16  (s = kt*128 + p)
        vv = np.ascontiguousarray(
            v[:, hs, :].transpose(1, 0, 2).reshape(HPC, NKT, 128, D)
            .transpose(0, 2, 1, 3)).astype(np.float16)
        in_maps.append({"qt": qt, "kt": kt, "v": vv})

    if trace:
        _install_ntff_hook()
    res = run_bass_kernel_spmd(nc, in_maps, core_ids=list(range(N_CORES)),
                               trace=trace, **trace_kwargs)

    out = np.empty((1, S, H, D), dtype=np.float32)
    for c in range(N_CORES):
        o = res.results[c]["out"].astype(np.float32)  # [HPC, NQC, D, QC]
        a = res.results[c]["acc"]    # [HPC, NQC, 128, 2, QC] f16
        den = a.astype(np.float32).sum(axis=(2, 3))  # [HPC, NQC, QC]
        on = o / den[:, :, None, :]                  # [HPC, NQC, D, QC]
        # -> [HPC, S, D]
        on = on.transpose(0, 1, 3, 2).reshape(HPC, S, D)
        for i in range(HPC):
            out[0, :, c * HPC + i, :] = on[i]
    if trace:
        kernel.last_results = res
    return out
